# revision 12
# baseline (speedup 1.0000x reference)
"""Trainium2 Bass kernel for nn_CausalFeatureTransformer.

Only the label row (row 128) of the transformer output is returned by the
reference, so the per-node computation collapses to:

  zn    = LN(Z[n])                                  (over 128 feats)
  s     = zn / sqrt(zn^2 * var_f + eps)             (per feature)
  score = G[h,j] * s[n,j] + D[h,j]                  (label-query attention)
  p     = softmax_j(score)   (max-free: |score| <~ 8)
  num_h = sum_j p*s*Cv_h[j]  den_h = sum_j e        (+ label-token consts)
  x     = (num/den) @ wo + c0
  y     = x + gelu(LN(x) @ w1' + b1') @ w2 + b2

where G, D, Cv, c0, w1', b1', ... are O(params) constants folded on the host
(weight preprocessing; independent of the batch data Z).

Sharding: pure data-parallel over nodes N: each of the 8 cores processes a
512-node shard of Z; the small folded params are replicated. The device
output is (64, 512) node-major-last (layout "B"); the host transposes on
gather.
"""

import math

import numpy as np

D_FEAT, D_EMB, H, DK = 128, 64, 4, 16
SEQ = D_FEAT + 1
N = 4096
N_CORES = 8
NS = N // N_CORES  # 512 nodes per core
EPS = 1e-5

_CACHE = {}


def _ln64(x, eps=EPS):
    m = x.mean(-1, keepdims=True)
    v = ((x - m) ** 2).mean(-1, keepdims=True)
    return (x - m) / np.sqrt(v + eps)


def _host_consts(A_full, feat_emb, label_token, wq, bq, wk, bk, wv, bv, wo, bo,
                 w1, b1, w2, b2, alpha, g1, be1, g2, be2):
    """Fold all O(params) quantities on the host (float64 for stability)."""
    d = np.float64
    fe = feat_emb.astype(d)
    mu = fe.mean(1, keepdims=True)
    vf = ((fe - mu) ** 2).mean(1)                    # (128,)
    cf = (fe - mu) * g1.astype(d)                    # (128,64)

    t = _ln64(label_token.astype(d)[0, 0]) * g1.astype(d) + be1.astype(d)
    qlab = t @ wq.astype(d) + bq.astype(d)
    klab = t @ wk.astype(d) + bk.astype(d)
    vlab = t @ wv.astype(d) + bv.astype(d)

    Ck = cf @ wk.astype(d)                           # (128,64)
    Cv = cf @ wv.astype(d)                           # (128,64)
    bk_p = be1.astype(d) @ wk.astype(d) + bk.astype(d)
    bv_p = be1.astype(d) @ wv.astype(d) + bv.astype(d)

    al = float(alpha)
    rdk = 1.0 / math.sqrt(DK)
    G = np.zeros((H, D_FEAT), d)
    Dm = np.zeros((H, D_FEAT), d)
    slab = np.zeros(H, d)
    for h in range(H):
        blk = slice(h * DK, (h + 1) * DK)
        G[h] = Ck[:, blk] @ qlab[blk] * rdk
        Dm[h] = qlab[blk] @ bk_p[blk] * rdk + al * A_full[:D_FEAT, D_FEAT].astype(d)
        slab[h] = qlab[blk] @ klab[blk] * rdk + al * A_full[D_FEAT, D_FEAT]
    elab = np.exp(slab)                              # (4,)

    c0 = label_token.astype(d)[0, 0] + bv_p @ wo.astype(d) + bo.astype(d)
    w1p = w1.astype(d) * g2.astype(d)[:, None]
    b1p = be2.astype(d) @ w1.astype(d) + b1.astype(d)
    # Per-head num lives at PSUM partition base 32h (matmul bases must be
    # 0/32/64/96): numbias row has the label-token num constants at 32h+k
    # (k<16), zero on the junk rows; denbias adds e_lab to every den copy;
    # wo_exp maps the strided num rows back to contiguous emb (junk rows x 0).
    nbias = elab[:, None] * (vlab - bv_p).reshape(H, DK)     # (4,16)
    numbias = np.zeros((2, 64))
    denbias = np.zeros((2, 64))
    wo_exp = np.zeros((2, 64, D_EMB))
    for h in range(H):
        pr, p = divmod(h, 2)
        numbias[pr, 32 * p:32 * p + DK] = nbias[h]
        denbias[pr, 32 * p:32 * p + 32] = elab[h]
        wo_exp[pr, 32 * p:32 * p + DK] = wo.astype(d)[h * DK:(h + 1) * DK]

    f = np.float32
    return {
        "sqrtvf": np.sqrt(vf).astype(f).reshape(D_FEAT, 1),
        "gmat": np.ascontiguousarray(G.T.astype(f)),        # (128,4)
        "dmat": np.ascontiguousarray(Dm.T.astype(f)),       # (128,4)
        "cv": Cv.astype(f),                                 # (128,64)
        "numbias_a": numbias[0:1].astype(f),                # (1,64)
        "numbias_b": numbias[1:2].astype(f),                # (1,64)
        "denbias_a": denbias[0:1].astype(f),                # (1,64)
        "denbias_b": denbias[1:2].astype(f),                # (1,64)
        "wo_a": np.ascontiguousarray(wo_exp[0].astype(f)),  # (64,64)
        "wo_b": np.ascontiguousarray(wo_exp[1].astype(f)),  # (64,64)
        "c0": c0.astype(f).reshape(D_EMB, 1),
        "w1p": w1p.astype(f),                               # (64,128)
        "b1p": b1p.astype(f).reshape(2 * D_EMB, 1),
        "w2mat": w2.astype(f),                               # (128,64)
        "b2c": b2.astype(f).reshape(D_EMB, 1),
    }


def _build_bass():
    import concourse.bacc as bacc
    import concourse.mybir as mybir
    import concourse.tile as tile
    from concourse.masks import make_identity

    f32 = mybir.dt.float32
    AF = mybir.ActivationFunctionType
    OP = mybir.AluOpType

    nc = bacc.Bacc("TRN2", target_bir_lowering=False, debug=False,
                   num_devices=N_CORES)

    zs = nc.dram_tensor("zs", (NS, D_FEAT), f32, kind="ExternalInput")
    consts = {}
    for name, shape in [
        ("sqrtvf", (D_FEAT, 1)), ("gmat", (D_FEAT, H)), ("dmat", (D_FEAT, H)),
        ("cv", (D_FEAT, D_EMB)),
        ("numbias_a", (1, 64)), ("numbias_b", (1, 64)),
        ("denbias_a", (1, 64)), ("denbias_b", (1, 64)),
        ("wo_a", (D_EMB, D_EMB)), ("wo_b", (D_EMB, D_EMB)),
        ("c0", (D_EMB, 1)), ("w1p", (D_EMB, 2 * D_EMB)),
        ("b1p", (2 * D_EMB, 1)), ("w2mat", (2 * D_EMB, D_EMB)), ("b2c", (D_EMB, 1)),
    ]:
        consts[name] = nc.dram_tensor(name, shape, f32, kind="ExternalInput")
    yt = nc.dram_tensor("yt", (D_EMB, NS), f32, kind="ExternalOutput")

    with tile.TileContext(nc) as tc:
        with (
            tc.tile_pool(name="cp", bufs=1) as cp,
            tc.tile_pool(name="wk", bufs=1) as wkp,
            tc.tile_pool(name="sm", bufs=2) as sm,
            tc.tile_pool(name="hd", bufs=3) as hd,
            tc.tile_pool(name="ps", bufs=1, space="PSUM") as ps,
        ):
            ct = {}
            for name, t in consts.items():
                ct[name] = cp.tile(list(t.shape), f32, tag=name, name=f"c_{name}")
                nc.sync.dma_start(out=ct[name], in_=t[:])
            ident = cp.tile([128, 128], f32, tag="ident")
            make_identity(nc, ident)
            ones_row = cp.tile([1, NS], f32, tag="ones_row")
            nc.vector.memset(ones_row, 1.0)
            eps_t = cp.tile([128, 1], f32, tag="eps_t")
            nc.vector.memset(eps_t, EPS)
            zero_t = cp.tile([128, 1], f32, tag="zero_t")
            nc.vector.memset(zero_t, 0.0)

            # ---- phase 1: LN(Z) rows + s, in layout A (node-part, feat-free)
            za = wkp.tile([128, 4, D_FEAT], f32, tag="za")
            nc.sync.dma_start(out=za, in_=zs.rearrange("(t p) f -> p t f", p=128))

            znT_ps = ps.tile([128, NS], f32, tag="psum0", name="znT_ps")
            for t in range(4):
                st6 = sm.tile([128, 6], f32, tag="st6")
                nc.vector.bn_stats(out=st6, in_=za[:, t, :])
                mv = sm.tile([128, 2], f32, tag="mv")
                nc.vector.bn_aggr(out=mv, in_=st6)
                std = sm.tile([128, 1], f32, tag="std")
                nc.scalar.activation(out=std, in_=mv[:, 1:2], func=AF.Sqrt, bias=eps_t)
                rstd = sm.tile([128, 1], f32, tag="rstd")
                nc.vector.reciprocal(out=rstd, in_=std)
                zn = sm.tile([128, D_FEAT], f32, tag="zn")
                nc.vector.tensor_scalar(
                    out=zn, in0=za[:, t, :], scalar1=mv[:, 0:1], scalar2=rstd,
                    op0=OP.subtract, op1=OP.mult)
                nc.tensor.transpose(znT_ps[:, t * 128:(t + 1) * 128], zn, ident)
            znT = wkp.tile([128, NS], f32, tag="znT")
            nc.scalar.copy(out=znT, in_=znT_ps)

            # s = zn / sqrt(zn^2 * vf + eps), layout B (feat-part, node-free)
            zsq = wkp.tile([128, NS], f32, tag="zsq")
            nc.scalar.activation(out=zsq, in_=znT, func=AF.Square, scale=ct["sqrtvf"], bias=zero_t)
            den0 = wkp.tile([128, NS], f32, tag="den0")
            nc.scalar.activation(out=den0, in_=zsq, func=AF.Sqrt, bias=eps_t)
            rr = wkp.tile([128, NS], f32, tag="rr")
            nc.vector.reciprocal(out=rr, in_=den0)
            sT = wkp.tile([128, NS], f32, tag="sT")
            nc.vector.tensor_mul(out=sT, in0=znT, in1=rr)

            # ---- attention (label query): per-head num at PSUM base 32h,
            # den replicated over 32 rows at base 32h of a second tile.
            ones32 = cp.tile([128, 32], f32, tag="ones32")
            nc.vector.memset(ones32, 1.0)
            num_psp = [ps.tile([D_EMB, NS], f32, tag="psum1", name="num_psa"),
                       ps.tile([D_EMB, NS], f32, tag="psum2", name="num_psb")]
            den_psp = [ps.tile([D_EMB, NS], f32, tag="psum3", name="den_psa"),
                       ps.tile([D_EMB, NS], f32, tag="psum4", name="den_psb")]
            for pr, sfx in ((0, "a"), (1, "b")):
                nc.tensor.matmul(num_psp[pr][:, :], ct[f"numbias_{sfx}"], ones_row,
                                 start=True, stop=False)
                nc.tensor.matmul(den_psp[pr][:, :], ct[f"denbias_{sfx}"], ones_row,
                                 start=True, stop=False)
            for h in range(4):
                pr, p = divmod(h, 2)
                eh = hd.tile([128, NS], f32, tag="eh", name="eh")
                nc.scalar.activation(out=eh, in_=sT, func=AF.Exp,
                                     scale=ct["gmat"][:, h:h + 1],
                                     bias=ct["dmat"][:, h:h + 1])
                esh = hd.tile([128, NS], f32, tag="esh", name="esh")
                nc.vector.tensor_mul(out=esh, in0=eh, in1=sT)
                nc.tensor.matmul(num_psp[pr][32 * p:32 * p + DK, :],
                                 ct["cv"][:, h * DK:(h + 1) * DK], esh,
                                 start=False, stop=(p == 1))
                nc.tensor.matmul(den_psp[pr][32 * p:32 * p + 32, :], ones32, eh,
                                 start=False, stop=(p == 1))

            x_ps = ps.tile([D_EMB, NS], f32, tag="psum5", name="x_ps")
            for pr, sfx in ((0, "a"), (1, "b")):
                rcp = wkp.tile([D_EMB, NS], f32, tag=f"rcp_{sfx}", name="rcp")
                nc.vector.reciprocal(out=rcp, in_=den_psp[pr])
                oe = wkp.tile([D_EMB, NS], f32, tag=f"oe_{sfx}", name="oe")
                nc.vector.tensor_mul(out=oe, in0=num_psp[pr], in1=rcp)
                nc.tensor.matmul(x_ps, ct[f"wo_{sfx}"], oe,
                                 start=(pr == 0), stop=(pr == 1))
            x_sb = wkp.tile([D_EMB, NS], f32, tag="x_sb")
            nc.scalar.activation(out=x_sb, in_=x_ps, func=AF.Identity, bias=ct["c0"])

            # ---- FFN layernorm: transpose x to layout A for row stats
            xa_ps = ps.tile([128, 4, D_EMB], f32, tag="psum0", name="xa_ps")
            for t in range(4):
                nc.tensor.transpose(xa_ps[:, t, :], x_sb[:, t * 128:(t + 1) * 128],
                                    ident[:64, :64])
            xa = wkp.tile([128, 4, D_EMB], f32, tag="xa")
            nc.scalar.copy(out=xa, in_=xa_ps)

            uT_ps = ps.tile([D_EMB, NS], f32, tag="psum1", name="uT_ps")
            for t in range(4):
                st6b = sm.tile([128, 6], f32, tag="st6b")
                nc.vector.bn_stats(out=st6b, in_=xa[:, t, :])
                mvb = sm.tile([128, 2], f32, tag="mvb")
                nc.vector.bn_aggr(out=mvb, in_=st6b)
                stdb = sm.tile([128, 1], f32, tag="stdb")
                nc.scalar.activation(out=stdb, in_=mvb[:, 1:2], func=AF.Sqrt, bias=eps_t)
                rstdb = sm.tile([128, 1], f32, tag="rstdb")
                nc.vector.reciprocal(out=rstdb, in_=stdb)
                uh = sm.tile([128, D_EMB], f32, tag="uh")
                nc.vector.tensor_scalar(
                    out=uh, in0=xa[:, t, :], scalar1=mvb[:, 0:1], scalar2=rstdb,
                    op0=OP.subtract, op1=OP.mult)
                nc.tensor.transpose(uT_ps[:, t * 128:(t + 1) * 128], uh, ident)
            uT = wkp.tile([D_EMB, NS], f32, tag="uT")
            nc.scalar.copy(out=uT, in_=uT_ps)

            # ---- FFN matmuls
            h_ps = ps.tile([2 * D_EMB, NS], f32, tag="psum2", name="h_ps")
            nc.tensor.matmul(h_ps, ct["w1p"], uT, start=True, stop=True)
            hh = wkp.tile([2 * D_EMB, NS], f32, tag="hh")
            nc.scalar.activation(out=hh, in_=h_ps, func=AF.Gelu, bias=ct["b1p"])
            y_ps = ps.tile([D_EMB, NS], f32, tag="psum3", name="y_ps")
            nc.tensor.matmul(y_ps, ct["w2mat"], hh, start=True, stop=False)
            nc.tensor.matmul(y_ps, ident[:64, :64], x_sb, start=False, stop=True)
            y_sb = wkp.tile([D_EMB, NS], f32, tag="y_sb")
            nc.scalar.activation(out=y_sb, in_=y_ps, func=AF.Identity, bias=ct["b2c"])

            nc.sync.dma_start(out=yt[:], in_=y_sb)

    nc.compile()
    return nc


def _get_nc():
    if "nc" not in _CACHE:
        _CACHE["nc"] = _build_bass()
    return _CACHE["nc"]


def kernel(Z, A_full, feat_emb, label_token, wq, bq, wk, bk, wv, bv, wo, bo,
           w1, b1, w2, b2, alpha, g1, be1, g2, be2, _trace=False, _trace_kwargs=None):
    from concourse.bass_utils import run_bass_kernel_spmd

    Z = np.ascontiguousarray(np.asarray(Z, dtype=np.float32))
    consts = _host_consts(
        np.asarray(A_full), np.asarray(feat_emb), np.asarray(label_token),
        np.asarray(wq), np.asarray(bq), np.asarray(wk), np.asarray(bk),
        np.asarray(wv), np.asarray(bv), np.asarray(wo), np.asarray(bo),
        np.asarray(w1), np.asarray(b1), np.asarray(w2), np.asarray(b2),
        np.asarray(alpha), np.asarray(g1), np.asarray(be1), np.asarray(g2),
        np.asarray(be2))
    consts = {k: np.ascontiguousarray(v) for k, v in consts.items()}

    nc = _get_nc()
    in_maps = []
    for c in range(N_CORES):
        m = dict(consts)
        m["zs"] = np.ascontiguousarray(Z[c * NS:(c + 1) * NS])
        in_maps.append(m)

    kw = {}
    if _trace:
        kw["trace"] = True
        if _trace_kwargs:
            kw.update(_trace_kwargs)
    res = run_bass_kernel_spmd(nc, in_maps, core_ids=list(range(N_CORES)), **kw)

    out = np.empty((N, D_EMB), np.float32)
    for c in range(N_CORES):
        out[c * NS:(c + 1) * NS] = res.results[c]["yt"].T
    if _trace:
        return out, res
    return out


# revision 14
# speedup vs baseline: 1.4467x; 1.4467x over previous
"""Trainium2 Bass kernel for nn_CausalFeatureTransformer.

Only the label row (row 128) of the transformer output is returned by the
reference, so the per-node computation collapses to:

  zn    = LN(Z[n])                                  (over 128 feats)
  s     = zn / sqrt(zn^2 * var_f + eps)             (per feature)
  score = G[h,j] * s[n,j] + D[h,j]                  (label-query attention)
  p     = softmax_j(score)   (max-free: |score| <~ 8)
  num_h = sum_j p*s*Cv_h[j]  den_h = sum_j e        (+ label-token consts)
  x     = (num/den) @ wo + c0
  y     = x + gelu(LN(x) @ w1' + b1') @ w2 + b2

where G, D, Cv, c0, w1', b1', ... are O(params) constants folded on the host
(weight preprocessing; independent of the batch data Z).

Sharding: pure data-parallel over nodes N: each of the 8 cores processes a
512-node shard of Z; the small folded params are replicated. The device
output is (64, 512) node-major-last; the host transposes on gather.

Device-side notes:
 - rsqrt is computed as Exp(-0.5*Ln(x)) so every ACT func used before the
   final Gelu lives in one activation-table set (no table reload thrash).
 - big reciprocals use the custom-DVE reciprocal_approx_fast (~18 bits).
 - per-head attention sums land at PSUM partition base 32*p (HW constraint:
   matmul out base must be 0/32/64); junk rows are zero-folded via wo.
 - matmul operands are bf16 (PSUM accumulation stays fp32).
"""

import math

import numpy as np

D_FEAT, D_EMB, H, DK = 128, 64, 4, 16
SEQ = D_FEAT + 1
N = 4096
N_CORES = 8
NS = N // N_CORES  # 512 nodes per core
EPS = 1e-5

_CACHE = {}


def _ln64(x, eps=EPS):
    m = x.mean(-1, keepdims=True)
    v = ((x - m) ** 2).mean(-1, keepdims=True)
    return (x - m) / np.sqrt(v + eps)


def _host_consts(A_full, feat_emb, label_token, wq, bq, wk, bk, wv, bv, wo, bo,
                 w1, b1, w2, b2, alpha, g1, be1, g2, be2):
    """Fold all O(params) quantities on the host (float64 for stability)."""
    import ml_dtypes
    d = np.float64
    fe = feat_emb.astype(d)
    mu = fe.mean(1, keepdims=True)
    vf = ((fe - mu) ** 2).mean(1)                    # (128,)
    cf = (fe - mu) * g1.astype(d)                    # (128,64)

    t = _ln64(label_token.astype(d)[0, 0]) * g1.astype(d) + be1.astype(d)
    qlab = t @ wq.astype(d) + bq.astype(d)
    klab = t @ wk.astype(d) + bk.astype(d)
    vlab = t @ wv.astype(d) + bv.astype(d)

    Ck = cf @ wk.astype(d)                           # (128,64)
    Cv = cf @ wv.astype(d)                           # (128,64)
    bk_p = be1.astype(d) @ wk.astype(d) + bk.astype(d)
    bv_p = be1.astype(d) @ wv.astype(d) + bv.astype(d)

    al = float(alpha)
    rdk = 1.0 / math.sqrt(DK)
    G = np.zeros((H, D_FEAT), d)
    Dm = np.zeros((H, D_FEAT), d)
    slab = np.zeros(H, d)
    for h in range(H):
        blk = slice(h * DK, (h + 1) * DK)
        G[h] = Ck[:, blk] @ qlab[blk] * rdk
        Dm[h] = qlab[blk] @ bk_p[blk] * rdk + al * A_full[:D_FEAT, D_FEAT].astype(d)
        slab[h] = qlab[blk] @ klab[blk] * rdk + al * A_full[D_FEAT, D_FEAT]
    elab = np.exp(slab)                              # (4,)

    c0 = label_token.astype(d)[0, 0] + bv_p @ wo.astype(d) + bo.astype(d)
    w1p = w1.astype(d) * g2.astype(d)[:, None]       # diag(g2) @ w1
    b1p = be2.astype(d) @ w1.astype(d) + b1.astype(d)

    # Head h lives in pair pr=h//2 at PSUM partition base 32*(h%2).
    nbias = elab[:, None] * (vlab - bv_p).reshape(H, DK)     # (4,16)
    numbias = np.zeros((2, 64))
    denbias = np.zeros((2, 64))
    wo_exp = np.zeros((128, D_EMB))                  # [pairA(64) ; pairB(64)]
    for h in range(H):
        pr, p = divmod(h, 2)
        numbias[pr, 32 * p:32 * p + DK] = nbias[h]
        denbias[pr, 32 * p:32 * p + 32] = elab[h]
        wo_exp[64 * pr + 32 * p:64 * pr + 32 * p + DK] = \
            wo.astype(d)[h * DK:(h + 1) * DK]

    f32 = np.float32
    bf16 = ml_dtypes.bfloat16

    # fp32 blob (128, 12): sqrtvf | G | D | b1p | c0 | b2c
    blob_f = np.zeros((128, 12), f32)
    blob_f[:, 0] = np.sqrt(vf)
    blob_f[:, 1:5] = G.T
    blob_f[:, 5:9] = Dm.T
    blob_f[:, 9] = b1p
    blob_f[:64, 10] = c0
    blob_f[:64, 11] = b2

    # bf16 blob (128, 320): cv | w2 | wo_exp | w1p (rows 0:64)
    blob_b = np.zeros((128, 320), bf16)
    blob_b[:, 0:64] = Cv.astype(bf16)
    blob_b[:, 64:128] = w2.astype(bf16)
    blob_b[:, 128:192] = wo_exp.astype(bf16)
    blob_b[:64, 192:320] = w1p.astype(bf16)

    # bf16 row blob (1, 256): numbias_a | numbias_b | denbias_a | denbias_b
    blob_r = np.zeros((1, 256), bf16)
    blob_r[0, 0:64] = numbias[0].astype(bf16)
    blob_r[0, 64:128] = numbias[1].astype(bf16)
    blob_r[0, 128:192] = denbias[0].astype(bf16)
    blob_r[0, 192:256] = denbias[1].astype(bf16)

    return {"blob_f": blob_f, "blob_b": blob_b, "blob_r": blob_r}


def _build_bass():
    import concourse.bacc as bacc
    import concourse.mybir as mybir
    import concourse.tile as tile
    from concourse.masks import make_identity

    f32 = mybir.dt.float32
    bf16 = mybir.dt.bfloat16
    AF = mybir.ActivationFunctionType
    OP = mybir.AluOpType

    nc = bacc.Bacc("TRN2", target_bir_lowering=False, debug=False,
                   num_devices=N_CORES)

    zs = nc.dram_tensor("zs", (NS, D_FEAT), f32, kind="ExternalInput")
    blob_f_d = nc.dram_tensor("blob_f", (128, 12), f32, kind="ExternalInput")
    blob_b_d = nc.dram_tensor("blob_b", (128, 320), bf16, kind="ExternalInput")
    blob_r_d = nc.dram_tensor("blob_r", (1, 256), bf16, kind="ExternalInput")
    yt = nc.dram_tensor("yt", (D_EMB, NS), f32, kind="ExternalOutput")

    with tile.TileContext(nc) as tc:
        with (
            tc.tile_pool(name="cp", bufs=1) as cp,
            tc.tile_pool(name="wk", bufs=1) as wkp,
            tc.tile_pool(name="sm", bufs=2) as sm,
            tc.tile_pool(name="hd", bufs=3) as hd,
            tc.tile_pool(name="ps", bufs=1, space="PSUM") as ps,
        ):
            # Z shard first on the sync queue so compute starts ASAP.
            za = wkp.tile([128, 4, D_FEAT], f32, tag="za")
            nc.sync.dma_start(out=za, in_=zs.rearrange("(t p) f -> p t f", p=128))
            bf = cp.tile([128, 12], f32, tag="bf", name="bf")
            nc.sync.dma_start(out=bf, in_=blob_f_d[:])
            bb = cp.tile([128, 320], bf16, tag="bb", name="bb")
            nc.sync.dma_start(out=bb, in_=blob_b_d[:])
            br = cp.tile([1, 256], bf16, tag="br", name="br")
            nc.sync.dma_start(out=br, in_=blob_r_d[:])

            sqrtvf = bf[:, 0:1]
            gcol = bf[:, 1:5]
            dcol = bf[:, 5:9]
            b1p = bf[:, 9:10]
            c0 = bf[:64, 10:11]
            b2c = bf[:64, 11:12]
            cv = bb[:, 0:64]
            w2m = bb[:, 64:128]
            wo_m = bb[:, 128:192]
            w1p = bb[:64, 192:320]

            ident = cp.tile([128, 128], f32, tag="ident")
            make_identity(nc, ident)
            identb = cp.tile([128, 128], bf16, tag="identb")
            make_identity(nc, identb)
            ones_row = cp.tile([1, NS], bf16, tag="ones_row")
            nc.vector.memset(ones_row, 1.0)
            ones32 = cp.tile([128, 32], bf16, tag="ones32")
            nc.vector.memset(ones32, 1.0)
            eps_t = cp.tile([128, 1], f32, tag="eps_t")
            nc.vector.memset(eps_t, EPS)

            # ---- phase 1: LN(Z) rows in layout A (node-part, feat-free)
            mvall = sm.tile([128, 4, 2], f32, tag="mvall", bufs=1)
            for t in range(4):
                st6 = sm.tile([128, 6], f32, tag="st6")
                nc.vector.bn_stats(out=st6, in_=za[:, t, :])
                nc.vector.bn_aggr(out=mvall[:, t, :], in_=st6)
            # rstd for all 4 t-blocks at once: exp(-0.5*ln(var+eps))
            lnv = sm.tile([128, 4], f32, tag="lnv", bufs=1)
            nc.scalar.activation(out=lnv, in_=mvall[:, :, 1], func=AF.Ln,
                                 bias=eps_t)
            rstd = sm.tile([128, 4], f32, tag="rstd", bufs=1)
            nc.scalar.activation(out=rstd, in_=lnv, func=AF.Exp, scale=-0.5)

            znT_ps = ps.tile([128, NS], f32, tag="psum0", name="znT_ps")
            for t in range(4):
                zn = sm.tile([128, D_FEAT], f32, tag="zn")
                nc.vector.tensor_scalar(
                    out=zn, in0=za[:, t, :], scalar1=mvall[:, t, 0:1],
                    scalar2=rstd[:, t:t + 1], op0=OP.subtract, op1=OP.mult)
                nc.tensor.transpose(znT_ps[:, t * 128:(t + 1) * 128], zn, ident)
            znT = wkp.tile([128, NS], f32, tag="znT")
            nc.scalar.copy(out=znT, in_=znT_ps)

            # ---- s = zn * rsqrt(zn^2*vf + eps), layout B (feat-part)
            zsq = wkp.tile([128, NS], f32, tag="zsq")
            nc.scalar.activation(out=zsq, in_=znT, func=AF.Square, scale=sqrtvf,
                                 bias=0.0)
            lns = wkp.tile([128, NS], f32, tag="lns")
            nc.scalar.activation(out=lns, in_=zsq, func=AF.Ln, bias=eps_t)
            rr = wkp.tile([128, NS], f32, tag="rr")
            nc.scalar.activation(out=rr, in_=lns, func=AF.Exp, scale=-0.5)
            sT = wkp.tile([128, NS], f32, tag="sT")
            nc.vector.tensor_mul(out=sT, in0=znT, in1=rr)

            # ---- attention (label query); head h -> pair pr=h//2, base 32p
            num_psp = [ps.tile([D_EMB, NS], f32, tag="psum1", name="num_psa"),
                       ps.tile([D_EMB, NS], f32, tag="psum2", name="num_psb")]
            den_psp = [ps.tile([D_EMB, NS], f32, tag="psum3", name="den_psa"),
                       ps.tile([D_EMB, NS], f32, tag="psum4", name="den_psb")]
            for pr in (0, 1):
                nc.tensor.matmul(num_psp[pr][:, :], br[:, 64 * pr:64 * pr + 64],
                                 ones_row, start=True, stop=False)
                nc.tensor.matmul(den_psp[pr][:, :],
                                 br[:, 128 + 64 * pr:192 + 64 * pr],
                                 ones_row, start=True, stop=False)
            for h in range(4):
                pr, p = divmod(h, 2)
                eh = hd.tile([128, NS], bf16, tag="eh", name="eh")
                nc.scalar.activation(out=eh, in_=sT, func=AF.Exp,
                                     scale=gcol[:, h:h + 1], bias=dcol[:, h:h + 1])
                esh = hd.tile([128, NS], bf16, tag="esh", name="esh")
                nc.vector.tensor_mul(out=esh, in0=eh, in1=sT)
                nc.tensor.matmul(num_psp[pr][32 * p:32 * p + DK, :],
                                 cv[:, h * DK:(h + 1) * DK], esh,
                                 start=False, stop=(p == 1))
                nc.tensor.matmul(den_psp[pr][32 * p:32 * p + 32, :], ones32, eh,
                                 start=False, stop=(p == 1))

            # oe = num/den for both pairs, stacked (128, NS) for one wo matmul
            oe = wkp.tile([128, NS], bf16, tag="oe")
            for pr in (0, 1):
                rcp = wkp.tile([D_EMB, NS], f32, tag=f"rcp{pr}", name="rcp")
                nc.vector.reciprocal_approx_fast(out=rcp, in_=den_psp[pr])
                nc.vector.tensor_mul(out=oe[64 * pr:64 * pr + 64, :],
                                     in0=num_psp[pr], in1=rcp)

            x_ps = ps.tile([D_EMB, NS], f32, tag="psum5", name="x_ps")
            nc.tensor.matmul(x_ps, wo_m, oe, start=True, stop=True)
            x_sb = wkp.tile([D_EMB, NS], f32, tag="x_sb")
            nc.scalar.activation(out=x_sb, in_=x_ps, func=AF.Identity, bias=c0)

            # ---- FFN layernorm via transpose to layout A
            xa_ps = ps.tile([128, 4, D_EMB], f32, tag="psum0", name="xa_ps")
            for t in range(4):
                nc.tensor.transpose(xa_ps[:, t, :], x_sb[:, t * 128:(t + 1) * 128],
                                    ident[:64, :64])
            xa = wkp.tile([128, 4, D_EMB], f32, tag="xa")
            nc.scalar.copy(out=xa, in_=xa_ps)

            mvb = sm.tile([128, 4, 2], f32, tag="mvb", bufs=1)
            for t in range(4):
                st6b = sm.tile([128, 6], f32, tag="st6b")
                nc.vector.bn_stats(out=st6b, in_=xa[:, t, :])
                nc.vector.bn_aggr(out=mvb[:, t, :], in_=st6b)
            lnvb = sm.tile([128, 4], f32, tag="lnvb", bufs=1)
            nc.scalar.activation(out=lnvb, in_=mvb[:, :, 1], func=AF.Ln,
                                 bias=eps_t)
            rstdb = sm.tile([128, 4], f32, tag="rstdb", bufs=1)
            nc.scalar.activation(out=rstdb, in_=lnvb, func=AF.Exp, scale=-0.5)

            uT_ps = ps.tile([D_EMB, NS], bf16, tag="psum1", name="uT_ps")
            for t in range(4):
                uh = sm.tile([128, D_EMB], bf16, tag="uh")
                nc.vector.tensor_scalar(
                    out=uh, in0=xa[:, t, :], scalar1=mvb[:, t, 0:1],
                    scalar2=rstdb[:, t:t + 1], op0=OP.subtract, op1=OP.mult)
                nc.tensor.transpose(uT_ps[:, t * 128:(t + 1) * 128], uh, identb)
            uT = wkp.tile([D_EMB, NS], bf16, tag="uT")
            nc.scalar.copy(out=uT, in_=uT_ps)

            # ---- FFN matmuls
            h_ps = ps.tile([2 * D_EMB, NS], f32, tag="psum2", name="h_ps")
            nc.tensor.matmul(h_ps, w1p, uT, start=True, stop=True)
            hh = wkp.tile([2 * D_EMB, NS], bf16, tag="hh")
            nc.scalar.activation(out=hh, in_=h_ps, func=AF.Gelu, bias=b1p)
            y_ps = ps.tile([D_EMB, NS], f32, tag="psum3", name="y_ps")
            nc.tensor.matmul(y_ps, w2m, hh, start=True, stop=True)
            yb = wkp.tile([D_EMB, NS], f32, tag="yb")
            nc.scalar.activation(out=yb, in_=y_ps, func=AF.Identity, bias=b2c)
            y_sb = wkp.tile([D_EMB, NS], f32, tag="y_sb")
            nc.vector.tensor_add(out=y_sb, in0=yb, in1=x_sb)

            nc.sync.dma_start(out=yt[:], in_=y_sb)

    nc.compile()
    return nc


def _get_nc():
    if "nc" not in _CACHE:
        _CACHE["nc"] = _build_bass()
    return _CACHE["nc"]


def kernel(Z, A_full, feat_emb, label_token, wq, bq, wk, bk, wv, bv, wo, bo,
           w1, b1, w2, b2, alpha, g1, be1, g2, be2, _trace=False,
           _trace_kwargs=None):
    from concourse.bass_utils import run_bass_kernel_spmd

    Z = np.ascontiguousarray(np.asarray(Z, dtype=np.float32))
    consts = _host_consts(
        np.asarray(A_full), np.asarray(feat_emb), np.asarray(label_token),
        np.asarray(wq), np.asarray(bq), np.asarray(wk), np.asarray(bk),
        np.asarray(wv), np.asarray(bv), np.asarray(wo), np.asarray(bo),
        np.asarray(w1), np.asarray(b1), np.asarray(w2), np.asarray(b2),
        np.asarray(alpha), np.asarray(g1), np.asarray(be1), np.asarray(g2),
        np.asarray(be2))
    consts = {k: np.ascontiguousarray(v) for k, v in consts.items()}

    nc = _get_nc()
    in_maps = []
    for c in range(N_CORES):
        m = dict(consts)
        m["zs"] = np.ascontiguousarray(Z[c * NS:(c + 1) * NS])
        in_maps.append(m)

    kw = {}
    if _trace:
        kw["trace"] = True
        if _trace_kwargs:
            kw.update(_trace_kwargs)
    res = run_bass_kernel_spmd(nc, in_maps, core_ids=list(range(N_CORES)), **kw)

    out = np.empty((N, D_EMB), np.float32)
    for c in range(N_CORES):
        out[c * NS:(c + 1) * NS] = res.results[c]["yt"].T
    if _trace:
        return out, res
    return out
